# revision 1
# baseline (speedup 1.0000x reference)
"""Trainium2 Bass kernel for CustomMLP: out = GELU(x@W1+b1)@W2 + b2.

x: (4, 2048, 1024) f32, W1: (1024, 4096), b1: (4096,), W2: (4096, 1024),
b2: (1024,). Data-parallel over the 8192 flattened rows: each of the 8
NeuronCores handles 1024 rows with fully replicated weights (no
collectives).

Per-core layout (everything transposed so both matmuls contract on the
partition axis with no on-chip transposes):
  xT   [1024(e), 1024(m)]           = x_shard^T
  hT   [h, m] computed on chip      (GELU applied on PSUM eviction)
  outT [1024(e2), 1024(m)]          host transposes back

matmul1: psum[h_blk, m] += w1[e_blk, h_blk].T @ xT[e_blk, m]
matmul2: psum[e2_blk, m] += w2[h_blk, e2_blk].T @ hT[h_blk, m]

Weights are host-packed so every DMA lands 4KB-contiguous per partition.
Matmuls run as float32r (full fp32 storage; 1 PE cycle/row at N>=256).
"""
import numpy as np

import concourse.bass as bass
import concourse.mybir as mybir
import concourse.tile as tile
from concourse import bacc
from concourse.bass_utils import run_bass_kernel_spmd

P = 128
N_CORES = 8

F32 = mybir.dt.float32
F32R = mybir.dt.float32r
GELU = mybir.ActivationFunctionType.Gelu
IDENT = mybir.ActivationFunctionType.Identity


def build_nc(M=1024, E=1024, H=4096, E2=1024, mm_dtype=F32R, act=GELU):
    """Build + compile the per-core program. M/E/H/E2 parameterized so a
    scaled-down version can run in CoreSim."""
    EB, HB, E2B = E // P, H // P, E2 // P
    MH = max(1, M // 512)  # m halves (moving-dim chunks of <=512)
    MS = M // MH           # moving chunk size
    W2Q = min(8, HB)       # h-blocks per w2 slab
    NQ = HB // W2Q         # w2 slabs per e2 block

    mmdt = mm_dtype
    nc = bacc.Bacc(None, target_bir_lowering=False)
    xT_d = nc.declare_dram_parameter("xT", [E, M], mmdt, isOutput=False)
    w1_d = nc.declare_dram_parameter("w1p", [HB, P, EB, P], mmdt, isOutput=False)
    b1_d = nc.declare_dram_parameter("b1p", [P, HB], F32, isOutput=False)
    w2_d = nc.declare_dram_parameter("w2p", [E2B, P, HB, P], mmdt, isOutput=False)
    b2_d = nc.declare_dram_parameter("b2p", [P, E2B], F32, isOutput=False)
    out_d = nc.declare_dram_parameter("outT", [E2B, P, M], F32, isOutput=True)

    xT_v = xT_d.rearrange("(eb p) m -> p eb m", p=P)

    with tile.TileContext(nc) as tc:
        with (
            tc.tile_pool(name="const", bufs=1) as cpool,
            tc.tile_pool(name="xp", bufs=1) as xpool,
            tc.tile_pool(name="hp", bufs=1) as hpool,
            tc.tile_pool(name="w1p", bufs=5) as w1pool,
            tc.tile_pool(name="w2p", bufs=4) as w2pool,
            tc.tile_pool(name="op", bufs=2) as opool,
            tc.tile_pool(name="ps1", bufs=3, space="PSUM") as psum1,
            tc.tile_pool(name="ps2", bufs=3, space="PSUM") as psum2,
        ):
            b1_sb = cpool.tile([P, HB], F32, name="b1s")
            b2_sb = cpool.tile([P, E2B], F32, name="b2s")
            nc.sync.dma_start(out=b1_sb[:], in_=b1_d[:])
            nc.sync.dma_start(out=b2_sb[:], in_=b2_d[:])

            # DMA queue is one FIFO: emit in consumption order. The first
            # HEAD_HBS h-blocks run m-half-major (all mh0 groups, then mh1)
            # so the PE's early xT demand rate is halved while the queue
            # ramps; w1 slabs for those blocks stay live across both halves.
            HEAD_HBS = min(4, HB)
            w1_tiles = {}
            w1_tiles[0] = w1pool.tile([P, EB, P], mmdt, name="w1t")
            nc.sync.dma_start(out=w1_tiles[0][:], in_=w1_d[0])

            xT_sb = xpool.tile([P, EB, M], mmdt, name="xT")
            for eb in range(EB):
                nc.sync.dma_start(out=xT_sb[:, eb, 0:MS], in_=xT_v[:, eb, 0:MS])
            for hb in range(1, HEAD_HBS):
                w1_tiles[hb] = w1pool.tile([P, EB, P], mmdt, name="w1t")
                nc.sync.dma_start(out=w1_tiles[hb][:], in_=w1_d[hb])
            for mh in range(1, MH):
                ms = slice(mh * MS, (mh + 1) * MS)
                for eb in range(EB):
                    nc.sync.dma_start(out=xT_sb[:, eb, ms], in_=xT_v[:, eb, ms])

            hT_sb = hpool.tile([P, HB, M], mmdt, name="hT")

            def mm1_group(w1_t, hb, mh):
                ms = slice(mh * MS, (mh + 1) * MS)
                ps = psum1.tile([P, MS], F32, name="ps1")
                for eb in range(EB):
                    nc.tensor.matmul(
                        ps[:],
                        lhsT=w1_t[:, eb, :],
                        rhs=xT_sb[:, eb, ms],
                        start=(eb == 0),
                        stop=(eb == EB - 1),
                    )
                nc.scalar.activation(
                    hT_sb[:, hb, ms], ps[:], act, bias=b1_sb[:, hb : hb + 1]
                )

            # ---- matmul 1 + GELU ----
            for mh in range(MH):
                for hb in range(HEAD_HBS):
                    mm1_group(w1_tiles[hb], hb, mh)
            for hb in range(HEAD_HBS, HB):
                w1_t = w1pool.tile([P, EB, P], mmdt, name="w1t")
                nc.sync.dma_start(out=w1_t[:], in_=w1_d[hb])
                for mh in range(MH):
                    mm1_group(w1_t, hb, mh)

            # ---- matmul 2 + bias ----
            for e2b in range(E2B):
                w2_ts = []
                for q in range(NQ):
                    w2_t = w2pool.tile([P, W2Q, P], mmdt, name="w2t")
                    nc.sync.dma_start(
                        out=w2_t[:], in_=w2_d[e2b, :, q * W2Q : (q + 1) * W2Q, :]
                    )
                    w2_ts.append(w2_t)
                out_sb = opool.tile([P, M], F32, name="outsb")
                for mh in range(MH):
                    ms = slice(mh * MS, (mh + 1) * MS)
                    ps2 = psum2.tile([P, MS], F32, name="ps2")
                    for hb in range(HB):
                        nc.tensor.matmul(
                            ps2[:],
                            lhsT=w2_ts[hb // W2Q][:, hb % W2Q, :],
                            rhs=hT_sb[:, hb, ms],
                            start=(hb == 0),
                            stop=(hb == HB - 1),
                        )
                    nc.scalar.activation(
                        out_sb[:, ms], ps2[:], IDENT, bias=b2_sb[:, e2b : e2b + 1]
                    )
                    nc.sync.dma_start(out=out_d[e2b, :, ms], in_=out_sb[:, ms])

    nc.compile()
    return nc


def pack_inputs(x, w1, b1, w2, b2):
    """Host-side shard + pack. Returns per-core input maps."""
    M_TOT = x.shape[0] * x.shape[1]
    E = x.shape[2]
    H = w1.shape[1]
    E2 = w2.shape[1]
    MC = M_TOT // N_CORES
    xf = np.ascontiguousarray(x.reshape(M_TOT, E))

    w1p = np.ascontiguousarray(
        w1.reshape(E // P, P, H // P, P).transpose(2, 1, 0, 3)
    )
    w2p = np.ascontiguousarray(
        w2.reshape(H // P, P, E2 // P, P).transpose(2, 1, 0, 3)
    )
    b1p = np.ascontiguousarray(b1.reshape(H // P, P).T)
    b2p = np.ascontiguousarray(b2.reshape(E2 // P, P).T)

    in_maps = []
    for i in range(N_CORES):
        xTi = np.ascontiguousarray(xf[i * MC : (i + 1) * MC].T)
        in_maps.append(
            {"xT": xTi, "w1p": w1p, "b1p": b1p, "w2p": w2p, "b2p": b2p}
        )
    return in_maps


def unpack_outputs(results, batch_shape=(4, 2048), E2=1024):
    M_TOT = batch_shape[0] * batch_shape[1]
    MC = M_TOT // N_CORES
    out = np.empty((M_TOT, E2), dtype=np.float32)
    for i in range(N_CORES):
        o = results[i]["outT"]  # [E2B, P, MC]
        out[i * MC : (i + 1) * MC] = o.transpose(2, 0, 1).reshape(MC, E2)
    return out.reshape(*batch_shape, E2)


_NC_CACHE = {}


def _get_nc():
    if "nc" not in _NC_CACHE:
        _NC_CACHE["nc"] = build_nc()
    return _NC_CACHE["nc"]


def kernel(x, w1, b1, w2, b2):
    nc = _get_nc()
    in_maps = pack_inputs(
        np.asarray(x, dtype=np.float32),
        np.asarray(w1, dtype=np.float32),
        np.asarray(b1, dtype=np.float32),
        np.asarray(w2, dtype=np.float32),
        np.asarray(b2, dtype=np.float32),
    )
    res = run_bass_kernel_spmd(nc, in_maps, core_ids=list(range(N_CORES))).results
    return unpack_outputs(res, batch_shape=(x.shape[0], x.shape[1]), E2=w2.shape[1])



# revision 2
# speedup vs baseline: 1.0719x; 1.0719x over previous
"""Trainium2 Bass kernel for CustomMLP: out = GELU(x@W1+b1)@W2 + b2.

x: (4, 2048, 1024) f32, W1: (1024, 4096), b1: (4096,), W2: (4096, 1024),
b2: (1024,). Data-parallel over the 8192 flattened rows: each of the 8
NeuronCores handles 1024 rows with fully replicated weights (no
collectives).

v2 (bf16): matmul operands are bf16 (host-cast); psum/bias/output stay
f32. Rel err ~3e-3 (CPU-sim'd) vs the 2e-2 gate. Changes vs the fp32r
baseline, all driven by the ntff trace:
  - DMA triggers cost ~610ns each on the issuing engine queue; the
    baseline's 98 triggers serialized ramp/stores. Now ~21 load
    triggers (w1 in 16 slabs of 2 h-blocks, x in 2 m-halves, w2 in 1,
    biases 2) with DRAM layouts host-packed so every trigger is a pure
    2D descriptor sweep (contiguous per-partition runs).
  - Loads go on the Sync HWDGE queue; output stores issue from the
    Activation queue so they stream during mm2 instead of clumping
    after the last matmul (11µs tail in baseline).
  - w2 (8MB bf16) is fully SBUF-resident, loaded during mm1: no
    per-e2b just-in-time w2 waits (was ~1-1.5µs stall per e2b).
  - PE warm-up: ~24 dummy N=128 matmuls on a zeroed tile run during
    the ~7µs DMA-init window so HAM un-throttles (1.2->2.4GHz) before
    real data lands.

Per-core layout (both matmuls contract on the partition axis):
  xT   [P, MH, EB, MS]  = x_shard^T, m-half-major
  hT   [P, HB, M]       on chip, GELU applied on PSUM eviction
  outT [E2B, P, M] f32  host transposes back
"""
import numpy as np
import ml_dtypes

import concourse.bass as bass
import concourse.mybir as mybir
import concourse.tile as tile
from concourse import bacc
from concourse.bass_utils import run_bass_kernel_spmd

P = 128
N_CORES = 8

F32 = mybir.dt.float32
BF16 = mybir.dt.bfloat16
GELU = mybir.ActivationFunctionType.Gelu
IDENT = mybir.ActivationFunctionType.Identity

SLAB = 2  # h-blocks per w1 slab
NWARM = 24


def build_nc(M=1024, E=1024, H=4096, E2=1024):
    EB, HB, E2B = E // P, H // P, E2 // P
    MH = max(1, M // 512)  # m halves (psum bank = 512 f32)
    MS = M // MH
    NS = HB // SLAB

    nc = bacc.Bacc(None, target_bir_lowering=False)
    xT_d = nc.declare_dram_parameter("xTp", [MH, P, EB, MS], BF16, isOutput=False)
    w1_d = nc.declare_dram_parameter("w1p", [NS, P, SLAB, EB, P], BF16, isOutput=False)
    b1_d = nc.declare_dram_parameter("b1p", [P, HB], F32, isOutput=False)
    w2_d = nc.declare_dram_parameter("w2p", [P, E2B, HB, P], BF16, isOutput=False)
    b2_d = nc.declare_dram_parameter("b2p", [P, E2B], F32, isOutput=False)
    out_d = nc.declare_dram_parameter("outT", [E2B, P, M], F32, isOutput=True)

    with tile.TileContext(nc) as tc:
        with (
            tc.tile_pool(name="const", bufs=1) as cpool,
            tc.tile_pool(name="w1p", bufs=4) as w1pool,
            tc.tile_pool(name="op", bufs=2) as opool,
            tc.tile_pool(name="ps1", bufs=3, space="PSUM") as psum1,
            tc.tile_pool(name="ps2", bufs=3, space="PSUM") as psum2,
            tc.tile_pool(name="psw", bufs=1, space="PSUM") as psumw,
        ):
            # ---- PE warm-up: matmuls on zeroed data while DMA init runs ----
            warm_sb = cpool.tile([P, P], BF16, name="warm")
            nc.gpsimd.memset(warm_sb[:], 0.0)
            psw_t = psumw.tile([P, P], F32, name="psw")
            for _ in range(NWARM):
                nc.tensor.matmul(
                    psw_t[:], lhsT=warm_sb[:], rhs=warm_sb[:], start=True, stop=True
                )

            # ---- loads: x + biases on the Activation queue, w1/w2 on Sync ----
            xT_sb = cpool.tile([P, MH, EB, MS], BF16, name="xT")
            b1_sb = cpool.tile([P, HB], F32, name="b1s")
            b2_sb = cpool.tile([P, E2B], F32, name="b2s")
            nc.scalar.dma_start(out=xT_sb[:, 0], in_=xT_d[0])
            nc.scalar.dma_start(out=b1_sb[:], in_=b1_d[:])
            nc.scalar.dma_start(out=b2_sb[:], in_=b2_d[:])
            for mh in range(1, MH):
                nc.scalar.dma_start(out=xT_sb[:, mh], in_=xT_d[mh])

            hT_sb = cpool.tile([P, HB, M], BF16, name="hT")
            w2_sb = cpool.tile([P, E2B, HB, P], BF16, name="w2s")

            def mm1_group(w1_t, i, hb, mh):
                ms = slice(mh * MS, (mh + 1) * MS)
                ps = psum1.tile([P, MS], F32, name="ps1")
                for eb in range(EB):
                    nc.tensor.matmul(
                        ps[:],
                        lhsT=w1_t[:, i, eb, :],
                        rhs=xT_sb[:, mh, eb, :],
                        start=(eb == 0),
                        stop=(eb == EB - 1),
                    )
                nc.scalar.activation(
                    hT_sb[:, hb, ms], ps[:], GELU, bias=b1_sb[:, hb : hb + 1]
                )

            # ---- matmul 1 + GELU ----
            # slab 0 runs m-half-major so its first groups only need the
            # first x m-half (halves the data the first matmul waits on).
            w1_t0 = w1pool.tile([P, SLAB, EB, P], BF16, name="w1t")
            nc.sync.dma_start(out=w1_t0[:], in_=w1_d[0])
            for mh in range(MH):
                for i in range(SLAB):
                    mm1_group(w1_t0, i, i, mh)
            for s in range(1, NS):
                if s == 4:
                    # w2 prefetch: single 8MB trigger, lands mid-mm1
                    nc.sync.dma_start(out=w2_sb[:], in_=w2_d[:])
                w1_t = w1pool.tile([P, SLAB, EB, P], BF16, name="w1t")
                nc.sync.dma_start(out=w1_t[:], in_=w1_d[s])
                for i in range(SLAB):
                    for mh in range(MH):
                        mm1_group(w1_t, i, s * SLAB + i, mh)

            # ---- matmul 2 + bias; stores stream from the Act queue ----
            for e2b in range(E2B):
                for mh in range(MH):
                    ms = slice(mh * MS, (mh + 1) * MS)
                    ps2 = psum2.tile([P, MS], F32, name="ps2")
                    for hb in range(HB):
                        nc.tensor.matmul(
                            ps2[:],
                            lhsT=w2_sb[:, e2b, hb, :],
                            rhs=hT_sb[:, hb, ms],
                            start=(hb == 0),
                            stop=(hb == HB - 1),
                        )
                    out_sb = opool.tile([P, MS], F32, name="outsb")
                    nc.scalar.activation(
                        out_sb[:], ps2[:], IDENT, bias=b2_sb[:, e2b : e2b + 1]
                    )
                    nc.scalar.dma_start(out=out_d[e2b, :, ms], in_=out_sb[:])

    nc.compile()
    return nc


def pack_inputs(x, w1, b1, w2, b2):
    """Host-side shard + pack (bf16 for matmul operands)."""
    M_TOT = x.shape[0] * x.shape[1]
    E = x.shape[2]
    H = w1.shape[1]
    E2 = w2.shape[1]
    MC = M_TOT // N_CORES
    EB, HB, E2B = E // P, H // P, E2 // P
    MH = max(1, MC // 512)
    MS = MC // MH
    NS = HB // SLAB
    bf = ml_dtypes.bfloat16

    xf = np.ascontiguousarray(x.reshape(M_TOT, E))

    # w1p[s, k, i, eb, m] = w1[eb*P+k, (s*SLAB+i)*P+m]
    w1p = np.ascontiguousarray(
        w1.reshape(EB, P, HB, P)
        .transpose(2, 1, 0, 3)
        .reshape(NS, SLAB, P, EB, P)
        .transpose(0, 2, 1, 3, 4)
        .astype(bf)
    )
    # w2p[k, e2b, hb, m] = w2[hb*P+k, e2b*P+m]
    w2p = np.ascontiguousarray(
        w2.reshape(HB, P, E2B, P).transpose(1, 2, 0, 3).astype(bf)
    )
    b1p = np.ascontiguousarray(b1.reshape(HB, P).T)
    b2p = np.ascontiguousarray(b2.reshape(E2B, P).T)

    in_maps = []
    for i in range(N_CORES):
        xc = xf[i * MC : (i + 1) * MC]  # [MC, E]
        # xTp[mh, p, eb, ms] = xc[mh*MS+ms, eb*P+p]
        xTp = np.ascontiguousarray(
            xc.reshape(MH, MS, EB, P).transpose(0, 3, 2, 1).astype(bf)
        )
        in_maps.append(
            {"xTp": xTp, "w1p": w1p, "b1p": b1p, "w2p": w2p, "b2p": b2p}
        )
    return in_maps


def unpack_outputs(results, batch_shape=(4, 2048), E2=1024):
    M_TOT = batch_shape[0] * batch_shape[1]
    MC = M_TOT // N_CORES
    out = np.empty((M_TOT, E2), dtype=np.float32)
    for i in range(N_CORES):
        o = results[i]["outT"]  # [E2B, P, MC]
        out[i * MC : (i + 1) * MC] = o.transpose(2, 0, 1).reshape(MC, E2)
    return out.reshape(*batch_shape, E2)


_NC_CACHE = {}


def _get_nc():
    if "nc" not in _NC_CACHE:
        _NC_CACHE["nc"] = build_nc()
    return _NC_CACHE["nc"]


def kernel(x, w1, b1, w2, b2):
    nc = _get_nc()
    in_maps = pack_inputs(
        np.asarray(x, dtype=np.float32),
        np.asarray(w1, dtype=np.float32),
        np.asarray(b1, dtype=np.float32),
        np.asarray(w2, dtype=np.float32),
        np.asarray(b2, dtype=np.float32),
    )
    res = run_bass_kernel_spmd(nc, in_maps, core_ids=list(range(N_CORES))).results
    return unpack_outputs(res, batch_shape=(x.shape[0], x.shape[1]), E2=w2.shape[1])


# revision 3
# speedup vs baseline: 1.0778x; 1.0055x over previous
"""Trainium2 Bass kernel for CustomMLP: out = GELU(x@W1+b1)@W2 + b2.

x: (4, 2048, 1024) f32, W1: (1024, 4096), b1: (4096,), W2: (4096, 1024),
b2: (1024,). Data-parallel over the 8192 flattened rows: each of the 8
NeuronCores handles 1024 rows with fully replicated weights (no
collectives). Matmul operands are bf16 (host-cast); psum/bias/output
stay f32 (rel err ~3.4e-3 vs the 2e-2 gate).

Trace-driven structure (ntff profile, v2 -> v3):
  - DMA triggers cost ~610ns on the issuing engine; loads are batched
    into ~2D triggers with host-packed DRAM layouts (contiguous
    per-partition runs): w1 in 16 slabs of 2 h-blocks (Sync queue),
    x in 4 m-quarters + biases (Activation queue), w2 in 8 per-e2b
    triggers spread across mm1 so no single 8MB transfer starves the
    ramp-critical x loads (that cost 10us of PE idle in v2).
  - Output stores issue from the Activation queue right after each
    bias-add so they stream during mm2 (v1 clumped 11us after the
    last matmul); the final store is split so the tail is ~1us.
  - First w1 slab is processed in m-quarter groups (N=256) so the
    first matmul only waits for 768KB of loads.
  - PE warm-up: 16 dummy N=512 matmuls on a zeroed tile run during
    the ~7us DMA-init window so HAM un-throttles (1.2->2.4GHz) before
    real data lands.
  - ps1 pool = 4 psum banks: at 3, start-of-group matmuls hit a
    ~430ns WAR stall on the GELU eviction every ~10.8us.

Per-core layout (both matmuls contract on the partition axis):
  xT   [P, EB, M]   = x_shard^T (loaded in m-quarters)
  hT   [P, HB, M]   on chip, GELU applied on PSUM eviction
  outT [E2B, P, M]  f32, host transposes back
"""
import numpy as np
import ml_dtypes

import concourse.bass as bass
import concourse.mybir as mybir
import concourse.tile as tile
from concourse import bacc
from concourse.bass_utils import run_bass_kernel_spmd

P = 128
N_CORES = 8

F32 = mybir.dt.float32
BF16 = mybir.dt.bfloat16
GELU = mybir.ActivationFunctionType.Gelu
IDENT = mybir.ActivationFunctionType.Identity

SLAB = 2   # h-blocks per w1 slab
MQ = 4     # x load quarters
NWARM = 16


def build_nc(M=1024, E=1024, H=4096, E2=1024):
    EB, HB, E2B = E // P, H // P, E2 // P
    MH = max(1, M // 512)  # m halves (psum bank = 512 f32)
    MS = M // MH
    MSQ = M // MQ
    NS = HB // SLAB

    nc = bacc.Bacc(None, target_bir_lowering=False)
    xT_d = nc.declare_dram_parameter("xTp", [MQ, P, EB, MSQ], BF16, isOutput=False)
    w1_d = nc.declare_dram_parameter("w1p", [NS, P, SLAB, EB, P], BF16, isOutput=False)
    b1_d = nc.declare_dram_parameter("b1p", [P, HB], F32, isOutput=False)
    w2_d = nc.declare_dram_parameter("w2p", [E2B, P, HB, P], BF16, isOutput=False)
    b2_d = nc.declare_dram_parameter("b2p", [P, E2B], F32, isOutput=False)
    out_d = nc.declare_dram_parameter("outT", [E2B, P, M], F32, isOutput=True)

    with tile.TileContext(nc) as tc:
        with (
            tc.tile_pool(name="const", bufs=1) as cpool,
            tc.tile_pool(name="w1p", bufs=4) as w1pool,
            tc.tile_pool(name="op", bufs=2) as opool,
            tc.tile_pool(name="ps1", bufs=4, space="PSUM") as psum1,
            tc.tile_pool(name="ps2", bufs=3, space="PSUM") as psum2,
            tc.tile_pool(name="psw", bufs=1, space="PSUM") as psumw,
        ):
            # ---- PE warm-up: matmuls on zeroed data while DMA init runs ----
            warm_sb = cpool.tile([P, 512], BF16, name="warm")
            nc.gpsimd.memset(warm_sb[:], 0.0)
            psw_t = psumw.tile([P, 512], F32, name="psw")
            for _ in range(NWARM):
                nc.tensor.matmul(
                    psw_t[:], lhsT=warm_sb[:, 0:P], rhs=warm_sb[:],
                    start=True, stop=True,
                )

            # ---- loads: x + biases on the Activation queue, w1/w2 on Sync ----
            xT_sb = cpool.tile([P, EB, M], BF16, name="xT")
            b1_sb = cpool.tile([P, HB], F32, name="b1s")
            b2_sb = cpool.tile([P, E2B], F32, name="b2s")
            for q in range(MQ):
                qs = slice(q * MSQ, (q + 1) * MSQ)
                nc.scalar.dma_start(out=xT_sb[:, :, qs], in_=xT_d[q])
                if q == 0:
                    nc.scalar.dma_start(out=b1_sb[:], in_=b1_d[:])
                    nc.scalar.dma_start(out=b2_sb[:], in_=b2_d[:])

            hT_sb = cpool.tile([P, HB, M], BF16, name="hT")
            w2_sb = cpool.tile([P, E2B, HB, P], BF16, name="w2s")

            def mm1_group(w1_t, i, hb, ms):
                ps = psum1.tile([P, ms.stop - ms.start], F32, name="ps1")
                for eb in range(EB):
                    nc.tensor.matmul(
                        ps[:],
                        lhsT=w1_t[:, i, eb, :],
                        rhs=xT_sb[:, eb, ms],
                        start=(eb == 0),
                        stop=(eb == EB - 1),
                    )
                nc.scalar.activation(
                    hT_sb[:, hb, ms], ps[:], GELU, bias=b1_sb[:, hb : hb + 1]
                )

            # ---- matmul 1 + GELU ----
            # slab 0 runs m-quarter-major so the first group only needs the
            # first x quarter (768KB of loads instead of 2.25MB).
            w1_t0 = w1pool.tile([P, SLAB, EB, P], BF16, name="w1t")
            nc.sync.dma_start(out=w1_t0[:], in_=w1_d[0])
            for q in range(MQ):
                for i in range(SLAB):
                    mm1_group(w1_t0, i, i, slice(q * MSQ, (q + 1) * MSQ))
            for s in range(1, NS):
                if 6 <= s < 6 + E2B:
                    # w2 prefetch: per-e2b triggers spread across mm1
                    e2b = s - 6
                    nc.sync.dma_start(out=w2_sb[:, e2b], in_=w2_d[e2b])
                w1_t = w1pool.tile([P, SLAB, EB, P], BF16, name="w1t")
                nc.sync.dma_start(out=w1_t[:], in_=w1_d[s])
                for i in range(SLAB):
                    for mh in range(MH):
                        mm1_group(
                            w1_t, i, s * SLAB + i, slice(mh * MS, (mh + 1) * MS)
                        )

            # ---- matmul 2 + bias; stores stream from the Act queue ----
            for e2b in range(E2B):
                for mh in range(MH):
                    ms = slice(mh * MS, (mh + 1) * MS)
                    ps2 = psum2.tile([P, MS], F32, name="ps2")
                    for hb in range(HB):
                        nc.tensor.matmul(
                            ps2[:],
                            lhsT=w2_sb[:, e2b, hb, :],
                            rhs=hT_sb[:, hb, ms],
                            start=(hb == 0),
                            stop=(hb == HB - 1),
                        )
                    out_sb = opool.tile([P, MS], F32, name="outsb")
                    nc.scalar.activation(
                        out_sb[:], ps2[:], IDENT, bias=b2_sb[:, e2b : e2b + 1]
                    )
                    if e2b == E2B - 1 and mh == MH - 1:
                        # split the final store so the post-matmul tail is
                        # one 256KB transfer, not 512KB
                        half = MS // 2
                        nc.scalar.dma_start(
                            out=out_d[e2b, :, ms.start : ms.start + half],
                            in_=out_sb[:, 0:half],
                        )
                        nc.scalar.dma_start(
                            out=out_d[e2b, :, ms.start + half : ms.stop],
                            in_=out_sb[:, half:MS],
                        )
                    else:
                        nc.scalar.dma_start(out=out_d[e2b, :, ms], in_=out_sb[:])

    nc.compile()
    return nc


def pack_inputs(x, w1, b1, w2, b2):
    """Host-side shard + pack (bf16 for matmul operands)."""
    M_TOT = x.shape[0] * x.shape[1]
    E = x.shape[2]
    H = w1.shape[1]
    E2 = w2.shape[1]
    MC = M_TOT // N_CORES
    EB, HB, E2B = E // P, H // P, E2 // P
    MSQ = MC // MQ
    NS = HB // SLAB
    bf = ml_dtypes.bfloat16

    xf = np.ascontiguousarray(x.reshape(M_TOT, E))

    # w1p[s, k, i, eb, m] = w1[eb*P+k, (s*SLAB+i)*P+m]
    w1p = np.ascontiguousarray(
        w1.reshape(EB, P, HB, P)
        .transpose(2, 1, 0, 3)
        .reshape(NS, SLAB, P, EB, P)
        .transpose(0, 2, 1, 3, 4)
        .astype(bf)
    )
    # w2p[e2b, k, hb, m] = w2[hb*P+k, e2b*P+m]
    w2p = np.ascontiguousarray(
        w2.reshape(HB, P, E2B, P).transpose(2, 1, 0, 3).astype(bf)
    )
    b1p = np.ascontiguousarray(b1.reshape(HB, P).T)
    b2p = np.ascontiguousarray(b2.reshape(E2B, P).T)

    in_maps = []
    for i in range(N_CORES):
        xc = xf[i * MC : (i + 1) * MC]  # [MC, E]
        # xTp[q, p, eb, ms] = xc[q*MSQ+ms, eb*P+p]
        xTp = np.ascontiguousarray(
            xc.reshape(MQ, MSQ, EB, P).transpose(0, 3, 2, 1).astype(bf)
        )
        in_maps.append(
            {"xTp": xTp, "w1p": w1p, "b1p": b1p, "w2p": w2p, "b2p": b2p}
        )
    return in_maps


def unpack_outputs(results, batch_shape=(4, 2048), E2=1024):
    M_TOT = batch_shape[0] * batch_shape[1]
    MC = M_TOT // N_CORES
    out = np.empty((M_TOT, E2), dtype=np.float32)
    for i in range(N_CORES):
        o = results[i]["outT"]  # [E2B, P, MC]
        out[i * MC : (i + 1) * MC] = o.transpose(2, 0, 1).reshape(MC, E2)
    return out.reshape(*batch_shape, E2)


_NC_CACHE = {}


def _get_nc():
    if "nc" not in _NC_CACHE:
        _NC_CACHE["nc"] = build_nc()
    return _NC_CACHE["nc"]


def kernel(x, w1, b1, w2, b2):
    nc = _get_nc()
    in_maps = pack_inputs(
        np.asarray(x, dtype=np.float32),
        np.asarray(w1, dtype=np.float32),
        np.asarray(b1, dtype=np.float32),
        np.asarray(w2, dtype=np.float32),
        np.asarray(b2, dtype=np.float32),
    )
    res = run_bass_kernel_spmd(nc, in_maps, core_ids=list(range(N_CORES))).results
    return unpack_outputs(res, batch_shape=(x.shape[0], x.shape[1]), E2=w2.shape[1])


# revision 4
# speedup vs baseline: 1.1064x; 1.0265x over previous
"""Trainium2 Bass kernel for CustomMLP: out = GELU(x@W1+b1)@W2 + b2.

x: (4, 2048, 1024) f32, W1: (1024, 4096), b1: (4096,), W2: (4096, 1024),
b2: (1024,). Data-parallel over the 8192 flattened rows: each of the 8
NeuronCores handles 1024 rows with fully replicated weights (no
collectives). Matmul operands are bf16 (host-cast); psum/bias/output
stay f32 (rel err ~3.4e-3 vs the 2e-2 gate).

Trace-driven structure (ntff profiles, v1->v4):
  - The PE matmul stream itself is the floor: 1024 N=512 bf16 matmuls
    at ~216ns issue-to-issue = 221us. Everything else is ramp/tail.
  - DMA triggers cost ~600ns on the issuing engine queue and HBM BW is
    ~310GB/s shared across queues, so the ramp is ordered for the
    first matmul group's exact needs on a single ring (Activation):
    x half 0 (1MB), w1 h-block 0 (256KB), h-block 1, biases, x half 1,
    then w1 slabs 2-3. Later w1 slabs go on the Sync ring, naturally
    time-gated past the ramp by the 4-buf pool WAR; w2 follows as 8
    per-e2b 1MB triggers spread across mm1 (a single 8MB trigger
    starved the ramp-critical x loads for 10us of PE idle in v2).
  - All DRAM layouts are host-packed so every trigger is a 2D
    descriptor sweep with >=2KB contiguous runs per partition.
  - Output stores issue from the Activation queue right after each
    bias-add so they stream during mm2 (v1 clumped 11us of stores
    after the last matmul).
  - PE warm-up: 10 dummy N=512 matmuls on a zeroed tile run during
    the ~7us engine-init window so HAM un-throttles (1.2->2.4GHz)
    before real data lands.
  - ps1 = 4 psum banks: at 3, start-of-group matmuls hit a ~430ns WAR
    stall against the GELU eviction.

Per-core layout (both matmuls contract on the partition axis):
  xT   [P, MH, EB, MS] = x_shard^T, m-half-major
  hT   [P, HB, M]      on chip, GELU applied on PSUM eviction
  outT [E2B, P, M]     f32, host transposes back
"""
import numpy as np
import ml_dtypes

import concourse.bass as bass
import concourse.mybir as mybir
import concourse.tile as tile
from concourse import bacc
from concourse.bass_utils import run_bass_kernel_spmd

P = 128
N_CORES = 8

F32 = mybir.dt.float32
BF16 = mybir.dt.bfloat16
GELU = mybir.ActivationFunctionType.Gelu
IDENT = mybir.ActivationFunctionType.Identity

SLAB = 2   # h-blocks per w1 slab (slabs 1..NS-1; slab 0 split per-hb)
NWARM = 10


def build_nc(M=1024, E=1024, H=4096, E2=1024):
    EB, HB, E2B = E // P, H // P, E2 // P
    MH = max(1, M // 512)  # m halves (psum bank = 512 f32)
    MS = M // MH
    NS = HB // SLAB

    nc = bacc.Bacc(None, target_bir_lowering=False)
    xT_d = nc.declare_dram_parameter("xTp", [MH, P, EB, MS], BF16, isOutput=False)
    w1_d = nc.declare_dram_parameter("w1p", [NS, P, SLAB, EB, P], BF16, isOutput=False)
    b1_d = nc.declare_dram_parameter("b1p", [P, HB], F32, isOutput=False)
    w2_d = nc.declare_dram_parameter("w2p", [E2B, P, HB, P], BF16, isOutput=False)
    b2_d = nc.declare_dram_parameter("b2p", [P, E2B], F32, isOutput=False)
    out_d = nc.declare_dram_parameter("outT", [E2B, P, M], F32, isOutput=True)

    with tile.TileContext(nc) as tc:
        with (
            tc.tile_pool(name="const", bufs=1) as cpool,
            tc.tile_pool(name="w1p", bufs=4) as w1pool,
            tc.tile_pool(name="op", bufs=2) as opool,
            tc.tile_pool(name="ps1", bufs=4, space="PSUM") as psum1,
            tc.tile_pool(name="ps2", bufs=3, space="PSUM") as psum2,
            tc.tile_pool(name="psw", bufs=1, space="PSUM") as psumw,
        ):
            # ---- PE warm-up: matmuls on zeroed data while DMA init runs ----
            warm_sb = cpool.tile([P, 512], BF16, name="warm")
            nc.gpsimd.memset(warm_sb[:], 0.0)
            psw_t = psumw.tile([P, 512], F32, name="psw")
            for _ in range(NWARM):
                nc.tensor.matmul(
                    psw_t[:], lhsT=warm_sb[:, 0:P], rhs=warm_sb[:],
                    start=True, stop=True,
                )

            # ---- ramp-critical loads, priority-ordered on the Act ring ----
            xT_sb = cpool.tile([P, MH, EB, MS], BF16, name="xT")
            b1_sb = cpool.tile([P, HB], F32, name="b1s")
            b2_sb = cpool.tile([P, E2B], F32, name="b2s")
            w1_ts = {}
            nc.scalar.dma_start(out=xT_sb[:, 0], in_=xT_d[0])
            for i in range(SLAB):
                w1_ts[i] = w1pool.tile([P, 1, EB, P], BF16, name="w1t")
                nc.scalar.dma_start(out=w1_ts[i][:], in_=w1_d[0, :, i : i + 1])
            nc.scalar.dma_start(out=b1_sb[:], in_=b1_d[:])
            nc.scalar.dma_start(out=b2_sb[:], in_=b2_d[:])
            for mh in range(1, MH):
                nc.scalar.dma_start(out=xT_sb[:, mh], in_=xT_d[mh])
            for s in (1, 2):
                w1_ts[s * SLAB] = w1pool.tile([P, SLAB, EB, P], BF16, name="w1t")
                nc.scalar.dma_start(out=w1_ts[s * SLAB][:], in_=w1_d[s])

            hT_sb = cpool.tile([P, HB, M], BF16, name="hT")
            w2_sb = cpool.tile([P, E2B, HB, P], BF16, name="w2s")

            def mm1_group(w1_t, i, hb, mh):
                ms = slice(mh * MS, (mh + 1) * MS)
                ps = psum1.tile([P, MS], F32, name="ps1")
                for eb in range(EB):
                    nc.tensor.matmul(
                        ps[:],
                        lhsT=w1_t[:, i, eb, :],
                        rhs=xT_sb[:, mh, eb, :],
                        start=(eb == 0),
                        stop=(eb == EB - 1),
                    )
                nc.scalar.activation(
                    hT_sb[:, hb, ms], ps[:], GELU, bias=b1_sb[:, hb : hb + 1]
                )

            # ---- matmul 1 + GELU ----
            # slab 0 runs m-half-major so the first group only waits on
            # x half 0 + one 256KB w1 block.
            for mh in range(MH):
                for i in range(SLAB):
                    mm1_group(w1_ts[i], 0, i, mh)
            for s in range(1, NS):
                if s >= 3:
                    if 6 <= s < 6 + E2B:
                        # w2 prefetch: per-e2b triggers spread across mm1
                        e2b = s - 6
                        nc.sync.dma_start(out=w2_sb[:, e2b], in_=w2_d[e2b])
                    w1_t = w1pool.tile([P, SLAB, EB, P], BF16, name="w1t")
                    nc.sync.dma_start(out=w1_t[:], in_=w1_d[s])
                else:
                    w1_t = w1_ts[s * SLAB]
                for i in range(SLAB):
                    for mh in range(MH):
                        mm1_group(w1_t, i, s * SLAB + i, mh)

            # ---- matmul 2 + bias; stores stream from the Act queue ----
            for e2b in range(E2B):
                for mh in range(MH):
                    ms = slice(mh * MS, (mh + 1) * MS)
                    ps2 = psum2.tile([P, MS], F32, name="ps2")
                    for hb in range(HB):
                        nc.tensor.matmul(
                            ps2[:],
                            lhsT=w2_sb[:, e2b, hb, :],
                            rhs=hT_sb[:, hb, ms],
                            start=(hb == 0),
                            stop=(hb == HB - 1),
                        )
                    out_sb = opool.tile([P, MS], F32, name="outsb")
                    nc.scalar.activation(
                        out_sb[:], ps2[:], IDENT, bias=b2_sb[:, e2b : e2b + 1]
                    )
                    nc.scalar.dma_start(out=out_d[e2b, :, ms], in_=out_sb[:])

    nc.compile()
    return nc


def pack_inputs(x, w1, b1, w2, b2):
    """Host-side shard + pack (bf16 for matmul operands)."""
    M_TOT = x.shape[0] * x.shape[1]
    E = x.shape[2]
    H = w1.shape[1]
    E2 = w2.shape[1]
    MC = M_TOT // N_CORES
    EB, HB, E2B = E // P, H // P, E2 // P
    MH = max(1, MC // 512)
    MS = MC // MH
    NS = HB // SLAB
    bf = ml_dtypes.bfloat16

    xf = np.ascontiguousarray(x.reshape(M_TOT, E))

    # w1p[s, k, i, eb, m] = w1[eb*P+k, (s*SLAB+i)*P+m]
    w1p = np.ascontiguousarray(
        w1.reshape(EB, P, HB, P)
        .transpose(2, 1, 0, 3)
        .reshape(NS, SLAB, P, EB, P)
        .transpose(0, 2, 1, 3, 4)
        .astype(bf)
    )
    # w2p[e2b, k, hb, m] = w2[hb*P+k, e2b*P+m]
    w2p = np.ascontiguousarray(
        w2.reshape(HB, P, E2B, P).transpose(2, 1, 0, 3).astype(bf)
    )
    b1p = np.ascontiguousarray(b1.reshape(HB, P).T)
    b2p = np.ascontiguousarray(b2.reshape(E2B, P).T)

    in_maps = []
    for i in range(N_CORES):
        xc = xf[i * MC : (i + 1) * MC]  # [MC, E]
        # xTp[mh, p, eb, ms] = xc[mh*MS+ms, eb*P+p]
        xTp = np.ascontiguousarray(
            xc.reshape(MH, MS, EB, P).transpose(0, 3, 2, 1).astype(bf)
        )
        in_maps.append(
            {"xTp": xTp, "w1p": w1p, "b1p": b1p, "w2p": w2p, "b2p": b2p}
        )
    return in_maps


def unpack_outputs(results, batch_shape=(4, 2048), E2=1024):
    M_TOT = batch_shape[0] * batch_shape[1]
    MC = M_TOT // N_CORES
    out = np.empty((M_TOT, E2), dtype=np.float32)
    for i in range(N_CORES):
        o = results[i]["outT"]  # [E2B, P, MC]
        out[i * MC : (i + 1) * MC] = o.transpose(2, 0, 1).reshape(MC, E2)
    return out.reshape(*batch_shape, E2)


_NC_CACHE = {}


def _get_nc():
    if "nc" not in _NC_CACHE:
        _NC_CACHE["nc"] = build_nc()
    return _NC_CACHE["nc"]


def kernel(x, w1, b1, w2, b2):
    nc = _get_nc()
    in_maps = pack_inputs(
        np.asarray(x, dtype=np.float32),
        np.asarray(w1, dtype=np.float32),
        np.asarray(b1, dtype=np.float32),
        np.asarray(w2, dtype=np.float32),
        np.asarray(b2, dtype=np.float32),
    )
    res = run_bass_kernel_spmd(nc, in_maps, core_ids=list(range(N_CORES))).results
    return unpack_outputs(res, batch_shape=(x.shape[0], x.shape[1]), E2=w2.shape[1])
